# revision 1
# baseline (speedup 1.0000x reference)
"""Trainium2 Bass kernel for MoE expert gating (nn_ExpertGating).

Computes, for x [16384, 4096], gate_w [64, 4096], gate_b [64]:
    g = x @ gate_w.T + gate_b                 # [16384, 64] fp32 logits
    top_vals, top_idx = top_k(g, k=8)         # descending, ties -> lowest index
    expert_weights = softmax(top_vals, -1)    # [16384, 8]
returns (expert_weights fp32 [16384, 8], top_idx int32 [16384, 8]).

Sharding: data-parallel over tokens -- 2048 tokens per core on 8 cores, the
64x4096 gate weight replicated. Per core the kernel streams x^T (pre-transposed
on host so the contraction dim H lands on SBUF partitions) through the PE in
32 H-chunks, accumulating logits [64 experts, tokens] in PSUM with full-fp32
matmuls (needed: min gap between 8th/9th logit is ~8e-6, so bf16/fp32r variants
would flip indices). Tokens are processed in 4 phases of 512 so each phase's
top-k tail overlaps the next phase's matmuls. Top-8 uses the DVE's native
max/max_index instructions; softmax is exp/sum/reciprocal on ACT+DVE.
"""

import numpy as np

import concourse.mybir as mybir
import concourse.tile as tile
from concourse import bacc
from concourse.bass_utils import run_bass_kernel_spmd

F32 = mybir.dt.float32
U32 = mybir.dt.uint32

NCORES = 8
T_FULL = 16384
H = 4096
E = 64
K = 8

T = T_FULL // NCORES          # 2048 tokens per core
NCHUNK = H // 128             # 32 contraction chunks
PHASES = 4
TP = T // PHASES              # 512 tokens per phase
TILES_P = TP // 128           # 4 token-tiles of 128 per phase
DMA_GROUPS = 4                # x DMAs per phase
CG = NCHUNK // DMA_GROUPS     # 8 chunks per DMA (2 MiB transfers)

_CACHE = {}


def _build():
    nc = bacc.Bacc("TRN2", target_bir_lowering=False, debug=False,
                   num_devices=NCORES)
    xt = nc.dram_tensor("xt", [H, T], F32, kind="ExternalInput").ap()
    wt = nc.dram_tensor("wt", [H, E], F32, kind="ExternalInput").ap()
    bias = nc.dram_tensor("bias", [E, 1], F32, kind="ExternalInput").ap()
    ident = nc.dram_tensor("ident", [E, E], F32, kind="ExternalInput").ap()
    w_out = nc.dram_tensor("w_out", [T, K], F32, kind="ExternalOutput").ap()
    i_out = nc.dram_tensor("i_out", [T, K], U32, kind="ExternalOutput").ap()

    with tile.TileContext(nc) as tc:
        with (
            tc.tile_pool(name="const", bufs=1) as cpool,
            tc.tile_pool(name="x", bufs=4) as xpool,
            tc.tile_pool(name="ps", bufs=2, space="PSUM") as pspool,
            tc.tile_pool(name="psT", bufs=2, space="PSUM") as psTpool,
            tc.tile_pool(name="lg", bufs=2) as lgpool,
            tc.tile_pool(name="sm", bufs=2) as smpool,
        ):
            wt_sb = cpool.tile([128, NCHUNK, E], F32)
            nc.sync.dma_start(wt_sb, wt.rearrange("(c p) e -> p c e", p=128))
            bias_sb = cpool.tile([E, 1], F32)
            nc.sync.dma_start(bias_sb, bias)
            ident_sb = cpool.tile([E, E], F32)
            nc.sync.dma_start(ident_sb, ident)

            for p in range(PHASES):
                # ---- gate matmul: logits [E, TP] += wt[c].T-chunks @ xt-chunks
                ps = pspool.tile([E, TP], F32)
                for g in range(DMA_GROUPS):
                    xt_sb = xpool.tile([128, CG, TP], F32)
                    src = xt[g * CG * 128:(g + 1) * CG * 128,
                             p * TP:(p + 1) * TP]
                    nc.sync.dma_start(
                        xt_sb, src.rearrange("(j q) t -> q j t", q=128))
                    for j in range(CG):
                        c = g * CG + j
                        nc.tensor.matmul(
                            ps,
                            lhsT=wt_sb[:, c, :],
                            rhs=xt_sb[:, j, :],
                            start=(c == 0),
                            stop=(c == NCHUNK - 1),
                        )

                # ---- bias add + move logits to SBUF
                logits_sb = lgpool.tile([E, TP], F32)
                nc.vector.tensor_scalar_add(logits_sb, ps, bias_sb)

                # ---- transpose to [tokens, experts]
                ltT = lgpool.tile([128, TILES_P, E], F32)
                for t in range(TILES_P):
                    psT = psTpool.tile([128, E], F32)
                    nc.tensor.transpose(
                        psT, logits_sb[:, t * 128:(t + 1) * 128], ident_sb)
                    nc.scalar.activation(
                        ltT[:, t, :], psT, mybir.ActivationFunctionType.Copy)

                # ---- hardware top-8 (descending) + indices
                vals = smpool.tile([128, TILES_P, K], F32)
                idxs = smpool.tile([128, TILES_P, K], U32)
                for t in range(TILES_P):
                    nc.vector.max(out=vals[:, t, :], in_=ltT[:, t, :])
                    nc.vector.max_index(out=idxs[:, t, :],
                                        in_max=vals[:, t, :],
                                        in_values=ltT[:, t, :])

                # ---- softmax over the 8 selected logits (batched per phase)
                sh = smpool.tile([128, TILES_P, K], F32)
                nc.vector.tensor_sub(
                    sh, vals, vals[:, :, 0:1].to_broadcast([128, TILES_P, K]))
                ex = smpool.tile([128, TILES_P, K], F32)
                nc.scalar.activation(ex, sh, mybir.ActivationFunctionType.Exp)
                sums = smpool.tile([128, TILES_P, 1], F32)
                nc.vector.reduce_sum(sums, ex, axis=mybir.AxisListType.X)
                rcp = smpool.tile([128, TILES_P, 1], F32)
                nc.vector.reciprocal(rcp, sums)
                wts = smpool.tile([128, TILES_P, K], F32)
                nc.vector.tensor_mul(
                    wts, ex, rcp.to_broadcast([128, TILES_P, K]))

                # ---- store phase outputs (token = p*TP + t*128 + partition)
                dst_w = w_out[p * TP:(p + 1) * TP, :].rearrange(
                    "(t q) k -> q t k", q=128)
                dst_i = i_out[p * TP:(p + 1) * TP, :].rearrange(
                    "(t q) k -> q t k", q=128)
                nc.sync.dma_start(dst_w, wts)
                nc.sync.dma_start(dst_i, idxs)

    nc.compile()
    return nc


def get_nc():
    if "nc" not in _CACHE:
        _CACHE["nc"] = _build()
    return _CACHE["nc"]


def make_in_maps(x, gate_w, gate_b):
    wt = np.ascontiguousarray(np.asarray(gate_w, dtype=np.float32).T)
    bias = np.asarray(gate_b, dtype=np.float32).reshape(E, 1).copy()
    ident = np.eye(E, dtype=np.float32)
    x = np.asarray(x, dtype=np.float32)
    in_maps = []
    for c in range(NCORES):
        xt = np.ascontiguousarray(x[c * T:(c + 1) * T].T)
        in_maps.append({"xt": xt, "wt": wt, "bias": bias, "ident": ident})
    return in_maps


def kernel(x, gate_w, gate_b):
    nc = get_nc()
    in_maps = make_in_maps(x, gate_w, gate_b)
    res = run_bass_kernel_spmd(nc, in_maps, core_ids=list(range(NCORES)))
    weights = np.concatenate(
        [res.results[c]["w_out"] for c in range(NCORES)], axis=0)
    idx = np.concatenate(
        [res.results[c]["i_out"] for c in range(NCORES)], axis=0)
    return weights, idx.astype(np.int32)
